# revision 25
# baseline (speedup 1.0000x reference)
"""Distributed GraphSAGE (2x SAGEConv + classifier) on 8 TRN2 NeuronCores.

Sharding: destination nodes are sharded 6250/core (the segment_sum outputs);
x is replicated so each core gathers x[src] locally; the small weights are
replicated. Between layers, chunked AllGathers share the hidden states so the
collective overlaps the tail of layer-1 compute.

Per-core pipeline (per layer):
  - edges (dst in this core's range) sorted by (dst-tile, src-half, src),
    padded per (tile, half) to multiples of 128 with SPMD-shared budgets
  - dma_gather pulls x[src] rows (int16 indices rebased per src-half),
    round-robined over 4 SWDGE queues
  - one-hot scatter matrices S[e, n] = (dst_rel[e] == n) built on VectorE
  - TensorE: msg[n, f] = sum_blocks S_blk.T @ Xg_blk (PSUM accumulate)
  - mean = msg * 1/max(deg, 1); hT = relu(W_l @ meanT + b + W_r @ xT)
  - layer 1: transpose hT back, DMA to h_local; chunked AllGather -> h_all
    (bf16, rank-major per chunk; layer-2 gather indices are host-remapped)
  - layer 2: classifier out = embT.T @ WcT + bc per tile
"""
import os

import ml_dtypes
import numpy as np

from concourse import bass, bacc, mybir, tile
from concourse.bass_utils import run_bass_kernel_spmd
from concourse.masks import make_identity

# problem constants (hardcoded per harness rules)
N = 50000
E = 640000
D = 128
NCLS = 64
CORES = 8
NSH = N // CORES          # 6250 nodes per core
P = 128
NT = (NSH + P - 1) // P   # 49 node tiles per core
HALF = N // 2             # src-half split for int16 gather indices
CH = int(os.environ.get("GNN_CH", 1024))  # gather chunk slots per dma_gather
NQ = int(os.environ.get("GNN_NQ", 4))     # swdge queues for gather DGE
NAG = int(os.environ.get("GNN_NAG", 4))   # chunked AllGather count
RING = int(os.environ.get("GNN_RING", 65536))  # SWDGE descriptor ring bytes
PADV = 200.0              # dst_rel pad value (never matches iota 0..127)

last_exec_ns = None


def configure(n, e):
    """Debug hook: shrink the problem (keeps D/NCLS/CORES)."""
    global N, E, NSH, NT, HALF
    N, E = n, e
    NSH = N // CORES
    NT = (NSH + P - 1) // P
    HALF = N // 2


def _ag_chunks():
    """Tile-range chunks for the chunked AllGather. Returns list of
    (tile_start, tile_end, row_start, row_end, out_offset)."""
    bounds = [round(k * NT / NAG) for k in range(NAG + 1)]
    chunks = []
    off = 0
    for k in range(NAG):
        t0, t1 = bounds[k], bounds[k + 1]
        r0, r1 = t0 * P, min(t1 * P, NSH)
        chunks.append((t0, t1, r0, r1, off))
        off += CORES * (r1 - r0)
    assert off == N
    return chunks


def _l2_remap():
    """Global node id -> row in the chunk-concatenated h_all buffer."""
    remap = np.empty(N, np.int64)
    for (t0, t1, r0, r1, off) in _ag_chunks():
        rk = r1 - r0
        for m in range(CORES):
            g0 = m * NSH + r0
            remap[g0:g0 + rk] = off + m * rk + np.arange(rk)
    return remap


# ----------------------------------------------------------------- host prep
def _prep_core(src, dst, m, budgets=None):
    """Extract + sort core m's edges (src already remapped for the layer).
    Returns per-(tile,half) counts or, given shared budgets, packed arrays."""
    sel = (dst >= m * NSH) & (dst < (m + 1) * NSH)
    s = src[sel].astype(np.int64)
    d = (dst[sel] - m * NSH).astype(np.int64)
    t = d >> 7
    half = (s >= HALF).astype(np.int64)
    order = np.lexsort((s, half, t))
    s, d, t, half = s[order], d[order], t[order], half[order]
    cnt = np.zeros((NT, 2), np.int64)
    np.add.at(cnt, (t, half), 1)
    if budgets is None:
        return cnt

    slots = budgets * P
    base = np.zeros((NT, 2), np.int64)
    for st in (0, 1):
        base[:, st] = np.concatenate(([0], np.cumsum(slots[:, st])[:-1]))
    g = t * 2 + half
    grp_cnt = np.zeros(NT * 2, np.int64)
    np.add.at(grp_cnt, g, 1)
    grp_start = np.concatenate(([0], np.cumsum(grp_cnt)[:-1]))
    rank = np.arange(len(s)) - grp_start[g]
    pos = base[t, half] + rank

    out = {}
    for st in (0, 1):
        L = int(slots[:, st].sum())
        idx = np.zeros(L, np.int64)
        drel = np.full(L, PADV, np.float32)
        msel = half == st
        idx[pos[msel]] = s[msel] - st * HALF
        drel[pos[msel]] = (d[msel] - (t[msel] << 7)).astype(np.float32)
        assert idx.max(initial=0) < 32768
        w16 = idx.astype(np.int16).reshape(L // 16, 16).T
        out[f"idx{st}"] = np.tile(w16, (CORES, 1)).copy()          # [128, L/16]
        out[f"drel{st}"] = drel.reshape(L // P, P).T.astype(ml_dtypes.bfloat16).copy()
    deg = np.bincount(d, minlength=NT * P).astype(np.int32)
    out["deg"] = deg.reshape(NT, P).T.copy()                       # [128, NT]
    return out


def _host_prep(x, edge_index, W1l, b1l, W1r, W2l, b2l, W2r, Wc, bc):
    src = np.asarray(edge_index[0], np.int64)
    dst = np.asarray(edge_index[1], np.int64)
    x = np.ascontiguousarray(np.asarray(x, np.float32))
    src_l2 = _l2_remap()[src]

    buds = []
    for lsrc in (src, src_l2):
        cnts = np.stack([_prep_core(lsrc, dst, m) for m in range(CORES)])
        buds.append(((cnts.max(axis=0) + P - 1) // P).astype(np.int64))

    iota = np.broadcast_to(np.arange(P, dtype=np.float32), (P, P)).copy()
    common = {
        "x_full": x.astype(ml_dtypes.bfloat16),
        "iota": iota.astype(ml_dtypes.bfloat16),
        "w1lt": np.ascontiguousarray(np.asarray(W1l, np.float32).T),
        "w1rt": np.ascontiguousarray(np.asarray(W1r, np.float32).T),
        "w2lt": np.ascontiguousarray(np.asarray(W2l, np.float32).T),
        "w2rt": np.ascontiguousarray(np.asarray(W2r, np.float32).T),
        "wct": np.ascontiguousarray(np.asarray(Wc, np.float32).T),
        "b1l": np.asarray(b1l, np.float32).reshape(D, 1).copy(),
        "b2l": np.asarray(b2l, np.float32).reshape(D, 1).copy(),
        "bcb": np.tile(np.asarray(bc, np.float32), (P, 1)).copy(),
    }
    in_maps = []
    for m in range(CORES):
        core = {}
        for lay, lsrc in enumerate((src, src_l2)):
            cm = _prep_core(lsrc, dst, m, buds[lay])
            for k, v in cm.items():
                if k != "deg" or lay == 0:
                    core[f"{k}_l{lay}" if k != "deg" else k] = v
        xT = np.zeros((D, NT * P), np.float32)
        xT[:, :NSH] = x[m * NSH:(m + 1) * NSH].T
        core["xt"] = xT.astype(ml_dtypes.bfloat16)
        core.update(common)
        in_maps.append(core)
    return in_maps, buds


# ------------------------------------------------------------- device build
def _build(nc: bacc.Bacc, buds):
    bf16 = mybir.dt.bfloat16
    f32 = mybir.dt.float32

    L_st = []
    base = []
    for lay in (0, 1):
        slots = buds[lay] * P
        L_st.append([int(slots[:, st].sum()) for st in (0, 1)])
        b = np.zeros((NT, 2), np.int64)
        for st in (0, 1):
            b[:, st] = np.concatenate(([0], np.cumsum(slots[:, st])[:-1]))
        base.append(b)

    # DRAM parameters
    x_full = nc.declare_dram_parameter("x_full", [N, D], bf16, isOutput=False)
    xt = nc.declare_dram_parameter("xt", [D, NT * P], bf16, isOutput=False)
    iota = nc.declare_dram_parameter("iota", [P, P], bf16, isOutput=False)
    idx_p, drel_p = {}, {}
    for lay in (0, 1):
        for st in (0, 1):
            idx_p[lay, st] = nc.declare_dram_parameter(
                f"idx{st}_l{lay}", [P, L_st[lay][st] // 16], mybir.dt.int16,
                isOutput=False)
            drel_p[lay, st] = nc.declare_dram_parameter(
                f"drel{st}_l{lay}", [P, L_st[lay][st] // P], bf16,
                isOutput=False)
    deg_p = nc.declare_dram_parameter("deg", [P, NT], mybir.dt.int32, isOutput=False)
    w_p = {k: nc.declare_dram_parameter(k, [D, D], f32, isOutput=False)
           for k in ("w1lt", "w1rt", "w2lt", "w2rt")}
    wct_p = nc.declare_dram_parameter("wct", [D, NCLS], f32, isOutput=False)
    b1l_p = nc.declare_dram_parameter("b1l", [D, 1], f32, isOutput=False)
    b2l_p = nc.declare_dram_parameter("b2l", [D, 1], f32, isOutput=False)
    bcb_p = nc.declare_dram_parameter("bcb", [P, NCLS], f32, isOutput=False)
    out_p = nc.declare_dram_parameter("out", [NSH, NCLS], f32, isOutput=True)

    h_local = nc.dram_tensor("h_local", [NSH, D], bf16)
    h_all = nc.dram_tensor("h_all", [N, D], bf16, addr_space="Shared")
    chunks = _ag_chunks()

    def bcast_mid(ap2d, nb):
        return bass.AP(ap2d.tensor, ap2d.offset, [ap2d.ap[0], [0, nb], list(ap2d.ap[1])])

    def bcast_last(ap2d, n):
        return bass.AP(ap2d.tensor, ap2d.offset, [ap2d.ap[0], list(ap2d.ap[1]), [0, n]])

    with tile.TileContext(nc) as tc:
        with (
            tc.tile_pool(name="cst", bufs=1) as cst,
            tc.tile_pool(name="sb", bufs=3) as sb,
            tc.tile_pool(name="xgp", bufs=6) as xgp,
            tc.tile_pool(name="xbp", bufs=8) as xbp,
            tc.tile_pool(name="sp", bufs=4) as spool,
            tc.tile_pool(name="ps", bufs=2, space="PSUM") as ps,
        ):
            # ---- constants ----
            iota_sb = cst.tile([P, P], bf16)
            nc.scalar.dma_start(out=iota_sb[:, :], in_=iota[:, :])
            ident = cst.tile([P, P], bf16)
            make_identity(nc, ident[:, :])

            wb = {}
            for k in ("w1lt", "w1rt", "w2lt", "w2rt"):
                wf = sb.tile([D, D], f32, tag="wload")
                nc.scalar.dma_start(out=wf[:, :], in_=w_p[k][:, :])
                wb[k] = cst.tile([D, D], bf16, tag=f"w_{k}", name=f"w_{k}")
                nc.vector.tensor_copy(wb[k][:, :], wf[:, :])
            wcf = sb.tile([D, NCLS], f32, tag="wload")
            nc.scalar.dma_start(out=wcf[:, :], in_=wct_p[:, :])
            wcb = cst.tile([D, NCLS], bf16)
            nc.vector.tensor_copy(wcb[:, :], wcf[:, :])

            b1l_sb = cst.tile([D, 1], f32)
            nc.scalar.dma_start(out=b1l_sb[:, :], in_=b1l_p[:, :])
            b2l_sb = cst.tile([D, 1], f32)
            nc.scalar.dma_start(out=b2l_sb[:, :], in_=b2l_p[:, :])
            bcb_sb = cst.tile([P, NCLS], f32)
            nc.scalar.dma_start(out=bcb_sb[:, :], in_=bcb_p[:, :])

            deg_i = sb.tile([P, NT], mybir.dt.int32, tag="degl")
            nc.scalar.dma_start(out=deg_i[:, :], in_=deg_p[:, :])
            deg_f = sb.tile([P, NT], f32, tag="degf")
            nc.vector.tensor_copy(deg_f[:, :], deg_i[:, :])
            nc.vector.tensor_scalar_max(deg_f[:, :], deg_f[:, :], 1.0)
            invdeg = cst.tile([P, NT], f32)
            nc.vector.reciprocal(invdeg[:, :], deg_f[:, :])

            xt_b = cst.tile([D, NT * P], bf16)
            nc.scalar.dma_start(out=xt_b[:, :], in_=xt[:, :])

            ht_b = cst.tile([D, NT * P], bf16)

            idx_sb, drel_sb = {}, {}
            for lay in (0, 1):
                for st in (0, 1):
                    it = cst.tile([P, L_st[lay][st] // 16], mybir.dt.int16,
                                  tag=f"idxsb{lay}{st}", name=f"idxsb{lay}{st}")
                    nc.scalar.dma_start(out=it[:, :], in_=idx_p[lay, st][:, :])
                    idx_sb[lay, st] = it
                    dt_ = cst.tile([P, L_st[lay][st] // P], bf16,
                                   tag=f"drelsb{lay}{st}", name=f"drelsb{lay}{st}")
                    nc.scalar.dma_start(out=dt_[:, :], in_=drel_p[lay, st][:, :])
                    drel_sb[lay, st] = dt_

            # ------------------------------------------------ one layer
            def do_layer(lay):
                src_half = (
                    [x_full[0:HALF, :], x_full[HALF:N, :]] if lay == 0
                    else [h_all[0:HALF, :], h_all[HALF:N, :]]
                )
                wl = wb["w1lt"] if lay == 0 else wb["w2lt"]
                wr = wb["w1rt"] if lay == 0 else wb["w2rt"]
                bias = b1l_sb if lay == 0 else b2l_sb
                rhs_loc = xt_b if lay == 0 else ht_b

                chunk_tiles = [{}, {}]
                qrr = [0]

                def get_chunk(st, c):
                    if c in chunk_tiles[st]:
                        return chunk_tiles[st][c]
                    ln = min(CH, L_st[lay][st] - c * CH)
                    idx_ap = idx_sb[lay, st][:, c * CH // 16:(c * CH + ln) // 16]
                    xb = xbp.tile([P, CH // P, D], bf16, tag=f"xb{st}")
                    nc.gpsimd.dma_gather(
                        out_ap=xb[:, :ln // P, :], in_ap=src_half[st],
                        idxs_ap=idx_ap, num_idxs=ln, num_idxs_reg=ln,
                        elem_size=D, single_packet=True, queue_num=qrr[0])
                    qrr[0] = (qrr[0] + 1) % NQ
                    chunk_tiles[st][c] = xb
                    return xb

                ag_iter = iter(chunks if lay == 0 else [])
                next_ag = next(ag_iter, None)

                for t in range(NT):
                    rows = min(P, NSH - t * P)
                    pm = ps.tile([P, D], f32, tag="msg")
                    blocks = []
                    for st in (0, 1):
                        nb = int(buds[lay][t, st])
                        if nb == 0:
                            continue
                        b0 = int(base[lay][t, st]) // P
                        S = spool.tile([P, nb, P], bf16, tag="s")
                        d_ap = drel_sb[lay, st][:, b0:b0 + nb]
                        nc.vector.tensor_tensor(
                            out=S[:, :, :], in0=bcast_mid(iota_sb[:, :], nb),
                            in1=bcast_last(d_ap, P), op=mybir.AluOpType.is_equal)
                        for b in range(nb):
                            slot = int(base[lay][t, st]) + b * P
                            xb = get_chunk(st, slot // CH)
                            blocks.append((S[:, b, :], xb[:, (slot % CH) // P, :]))
                    if not blocks:
                        nc.vector.memset(pm[:, :], 0.0)
                    for i, (s_ap, x_ap) in enumerate(blocks):
                        nc.tensor.matmul(pm[:, :], lhsT=s_ap, rhs=x_ap,
                                         start=(i == 0), stop=(i == len(blocks) - 1))

                    mean_b = sb.tile([P, D], bf16, tag="mean")
                    nc.vector.tensor_scalar(
                        out=mean_b[:, :], in0=pm[:, :],
                        scalar1=invdeg[:, t:t + 1], scalar2=None,
                        op0=mybir.AluOpType.mult)
                    pt = ps.tile([P, D], bf16, tag="tr")
                    nc.tensor.transpose(pt[:, :], mean_b[:, :], ident[:, :])
                    meanT = sb.tile([P, D], bf16, tag="meanT")
                    nc.scalar.activation(meanT[:, :], pt[:, :],
                                         mybir.ActivationFunctionType.Copy)

                    ph = ps.tile([D, P], f32, tag="hT")
                    nc.tensor.matmul(ph[:, :], lhsT=wl[:, :], rhs=meanT[:, :],
                                     start=True, stop=False)
                    nc.tensor.matmul(ph[:, :], lhsT=wr[:, :],
                                     rhs=rhs_loc[:, t * P:(t + 1) * P],
                                     start=False, stop=True)

                    if lay == 0:
                        hT = ht_b[:, t * P:(t + 1) * P]
                        nc.scalar.activation(hT, ph[:, :],
                                             mybir.ActivationFunctionType.Relu,
                                             bias=bias[:, :])
                        phn = ps.tile([P, D], bf16, tag="aux")
                        nc.tensor.transpose(phn[:, :], hT, ident[:, :])
                        h_sb = sb.tile([P, D], bf16, tag="hs")
                        nc.vector.tensor_copy(h_sb[:, :], phn[:, :])
                        nc.scalar.dma_start(out=h_local[t * P:t * P + rows, :],
                                            in_=h_sb[:rows, :])
                        if next_ag is not None and t == next_ag[1] - 1:
                            t0, t1, r0, r1, off = next_ag
                            rk = r1 - r0
                            nc.gpsimd.collective_compute(
                                "AllGather", mybir.AluOpType.bypass,
                                replica_groups=[list(range(CORES))],
                                ins=[h_local[r0:r1, :].opt()],
                                outs=[h_all[off:off + CORES * rk, :].opt()])
                            next_ag = next(ag_iter, None)
                    else:
                        embT = sb.tile([D, P], bf16, tag="embT")
                        nc.scalar.activation(embT[:, :], ph[:, :],
                                             mybir.ActivationFunctionType.Relu,
                                             bias=bias[:, :])
                        pc = ps.tile([P, NCLS], f32, tag="aux")
                        nc.tensor.matmul(pc[:, :], lhsT=embT[:, :], rhs=wcb[:, :],
                                         start=True, stop=True)
                        oc = sb.tile([P, NCLS], f32, tag="oc")
                        nc.vector.tensor_tensor(out=oc[:, :], in0=pc[:, :],
                                                in1=bcb_sb[:, :],
                                                op=mybir.AluOpType.add)
                        nc.scalar.dma_start(out=out_p[t * P:t * P + rows, :],
                                            in_=oc[:rows, :])

            do_layer(0)
            do_layer(1)
    return nc


# ------------------------------------------------------------------- driver
def _enable_axon_trace():
    """The agent image's antenv lacks axon_hooks; synthesize it from the
    ctypes NTFF hook in trn_agent_boot so trace=True works under axon."""
    import sys
    import types
    try:
        import antenv.axon_hooks  # noqa: F401
        return True
    except ImportError:
        pass
    try:
        from trn_agent_boot.trn_boot import _ntff_profile_via_ctypes
        hook = _ntff_profile_via_ctypes("/opt/axon/libaxon_pjrt.so")
        if hook is None:
            return False
        mod = types.ModuleType("antenv.axon_hooks")
        mod.get_axon_ntff_profile_hook = lambda: hook
        mod.set_axon_ntff_profile_hook = lambda h: None
        sys.modules["antenv.axon_hooks"] = mod
        from concourse import bass_utils as _bu
        _bu.upload_artifacts = lambda tmpdir: f"file://{tmpdir}"
        return True
    except Exception:
        return False


def kernel(x, edge_index, W1l, b1l, W1r, W2l, b2l, W2r, Wc, bc):
    global last_exec_ns
    in_maps, buds = _host_prep(x, edge_index, W1l, b1l, W1r, W2l, b2l, W2r,
                               Wc, bc)
    nc = _build(bacc.Bacc(num_swdge_queues=NQ, dynamic_dma_scratch_size=RING), buds)
    nc.compile()
    trace = os.environ.get("GNN_TRACE", "0") == "1" and _enable_axon_trace()
    r = run_bass_kernel_spmd(nc, in_maps, core_ids=list(range(CORES)),
                             trace=trace)
    last_exec_ns = r.exec_time_ns
    out = np.concatenate([r.results[m]["out"] for m in range(CORES)], axis=0)
    return out.astype(np.float32)

